# revision 9
# baseline (speedup 1.0000x reference)
"""GCNConvSC (residual + GCNConv) Trainium2 Bass kernel, 8-core SPMD.

Math (matches the PyG-style reference):
    deg[v]  = indeg_with_selfloop(v)          (count of v in dst, +1)
    u       = deg^{-1/2}
    y       = u[:,None] * x                   (pre-scaled node features)
    z[v]    = sum_{e: dst_e = v} y[src_e]     (unweighted edge aggregation)
    out[v]  = x[v] + b + (u[v] * (z[v] + y[v])) @ W

The per-edge norm u[src]*u[dst] factorizes: u[src] folds into y (gather
source), u[dst] is a post-aggregation row scale, and the self-loop term
u[v]^2*x[v] is the acc's ys initialization. The matmul by W commutes with
the segment-sum, so it runs once per node after aggregation.

Sharding: destination nodes are range-partitioned over the 8 cores
(12544 dst slots per core). Each core gathers y[src] rows for its edges
from a replicated y in its HBM via dma_gather (int16 indices => 4 source
chunks of 25024 rows), and aggregates them with one-hot matmuls on the
tensor engine into PSUM windows of 128 dst slots (feat-major), 4 windows
per PSUM bank. The one-hot [128 edges x 128 slots] for each edge tile is
built on the vector engine as (iota == slot) with a staged iota tile.
Edges are sorted by (window-group, src-chunk, window) on the host and
padded per (chunk, window) run to multiples of 128 so every matmul is
window-pure; pad edges use src index 0 with slot -1 (one-hot row = 0).

The schedule (tile counts per (group, chunk, window)) is shared across
all 8 cores (SPMD single program), using the max count over cores.
"""

import sys

sys.path.insert(0, "/opt/trn_rl_repo")

import numpy as np

N_NODES = 100000
F = 128
N_CORES = 8
S = 12544            # dst slots per core (98 windows of 128)
WN = 98              # windows per core
WG_SIZE = 4          # windows per PSUM bank group
N_CHUNKS = 4
CHUNK = 25024        # gather-source rows per chunk (int16-safe)
NPAD = N_CHUNKS * CHUNK  # 100096 padded node rows for y

MSGS_DT = "float32"  # dtype of gathered messages (y) and matmul lhsT
OH_DT = "float32"    # dtype of iota/slots/one-hot (matmul rhs)


def _host_plan(edge_index):
    """Sort/bucket edges per core; emit the shared SPMD schedule plus
    per-core gather-index and slot arrays."""
    src = np.asarray(edge_index[0], dtype=np.int64)
    dst = np.asarray(edge_index[1], dtype=np.int64)

    deg = np.bincount(dst, minlength=N_NODES).astype(np.float64) + 1.0
    u = (1.0 / np.sqrt(deg)).astype(np.float32)

    core_of = dst // S
    chunk_of = src // CHUNK

    # per-core, per-(window, chunk) edge lists
    per_core = []
    counts = np.zeros((N_CORES, N_CHUNKS, WN), dtype=np.int64)
    for c in range(N_CORES):
        m = core_of == c
        es, ed = src[m], dst[m] - c * S
        ch = chunk_of[m]
        w = ed // 128
        # sort edges by (window-group, chunk, window)
        wg = w // WG_SIZE
        order = np.lexsort((w, ch, wg))
        es, ed, ch, w = es[order], ed[order], ch[order], w[order]
        np.add.at(counts[c], (ch, w), 1)
        per_core.append((es, ed, ch, w))

    # shared schedule: tiles per (chunk, window) = max over cores
    n_tiles = np.maximum((counts.max(axis=0) + 127) // 128, 0)  # [N_CHUNKS, WN]
    # every window needs >=1 tile overall so its PSUM quarter gets written
    empty_w = n_tiles.sum(axis=0) == 0
    n_tiles[0, empty_w] = 1

    # global tile order: for wg, for chunk, for window in wg
    n_wg = (WN + WG_SIZE - 1) // WG_SIZE
    sched = []  # list of segments: (chunk, [(window, q, ntiles, first, last)])
    T = 0
    for g in range(n_wg):
        ws = range(g * WG_SIZE, min((g + 1) * WG_SIZE, WN))
        touched = [w for w in ws if n_tiles[:, w].sum() > 0]
        first_touch = {w: None for w in touched}
        last_touch = {w: None for w in touched}
        segs = []
        for ch in range(N_CHUNKS):
            tl = []
            for w in ws:
                nt = int(n_tiles[ch, w])
                if nt == 0:
                    continue
                tl.append([w, w % WG_SIZE, nt])
                if first_touch[w] is None:
                    first_touch[w] = (ch, w)
                last_touch[w] = (ch, w)
            segs.append(tl)
        sched.append((g, segs, first_touch, last_touch))
        T += int(n_tiles[:, list(ws)].sum())

    # per-core padded edge streams in schedule order
    idx16 = np.zeros((N_CORES, T * 128), dtype=np.int16)
    slots = np.full((N_CORES, T * 128), -1.0, dtype=np.float32)
    for c in range(N_CORES):
        es, ed, ch, w = per_core[c]
        # bucket start offsets within the sorted arrays
        pos = 0
        # edges are sorted by (wg, chunk, window); walk in the same order
        ptr = {}
        start = 0
        keys = list(zip(w // WG_SIZE, ch, w))
        # compute run starts
        run_start = {}
        for i, k in enumerate(keys):
            if k not in run_start:
                run_start[k] = i
        run_len = counts[c]
        out_pos = 0
        for g, segs, _, _ in sched:
            for chp in range(N_CHUNKS):
                for wseg, q, nt in segs[chp]:
                    cnt = int(run_len[chp, wseg])
                    if cnt > 0:
                        i0 = run_start[(g, chp, wseg)]
                        sl = slice(i0, i0 + cnt)
                        local = (es[sl] - chp * CHUNK).astype(np.int16)
                        idx16[c, out_pos : out_pos + cnt] = local
                        slots[c, out_pos : out_pos + cnt] = (ed[sl] % 128).astype(
                            np.float32
                        )
                    out_pos += nt * 128
        assert out_pos == T * 128

    return u, n_tiles, sched, T, idx16, slots


def _build_program(T, sched):
    import concourse.bacc as bacc
    import concourse.mybir as mybir
    from concourse import tile

    dt = getattr(mybir.dt, MSGS_DT)
    oh_dt = getattr(mybir.dt, OH_DT)
    f32 = mybir.dt.float32

    nc = bacc.Bacc(
        "TRN2",
        target_bir_lowering=False,
        debug=False,
        enable_asserts=True,
        num_devices=N_CORES,
    )

    y_d = nc.dram_tensor("y", [NPAD, F], dt, kind="ExternalInput").ap()
    idx_d = nc.dram_tensor("idx16", [128, T * 8], mybir.dt.int16, kind="ExternalInput").ap()
    slots_d = nc.dram_tensor("slots", [128, T], oh_dt, kind="ExternalInput").ap()
    iota_d = nc.dram_tensor("iota", [128, 128], oh_dt, kind="ExternalInput").ap()
    usT_d = nc.dram_tensor("usT", [128, S], f32, kind="ExternalInput").ap()
    ysT_d = nc.dram_tensor("ysT", [128, S], f32, kind="ExternalInput").ap()
    xsT_d = nc.dram_tensor("xsT", [128, S], f32, kind="ExternalInput").ap()
    w_d = nc.dram_tensor("W", [F, F], f32, kind="ExternalInput").ap()
    out_d = nc.dram_tensor("outT", [128, S], f32, kind="ExternalOutput").ap()

    with tile.TileContext(nc) as tc:
        with (
            tc.tile_pool(name="const", bufs=1) as const_p,
            tc.tile_pool(name="acc", bufs=1) as acc_p,
            tc.tile_pool(name="msgs", bufs=3) as msgs_p,
            tc.tile_pool(name="oh", bufs=4) as oh_p,
            tc.tile_pool(name="psum", bufs=6, space="PSUM") as psum_p,
            tc.tile_pool(name="fin", bufs=2) as fin_p,
            tc.tile_pool(name="fpsum", bufs=2, space="PSUM") as fpsum_p,
        ):
            idx_sb = const_p.tile([128, T * 8], mybir.dt.int16)
            slots_sb = const_p.tile([128, T], oh_dt)
            iota_sb = const_p.tile([128, 128], oh_dt)
            w_sb = const_p.tile([F, F], f32)
            acc = acc_p.tile([128, S], f32)

            nc.sync.dma_start(idx_sb[:], idx_d[:])
            nc.sync.dma_start(slots_sb[:], slots_d[:])
            nc.sync.dma_start(iota_sb[:], iota_d[:])
            nc.sync.dma_start(w_sb[:], w_d[:])
            # acc starts as ys^T (self-loop term y[v], scaled later by u[v])
            nc.sync.dma_start(acc[:], ysT_d[:])

            g_tile = 0  # global tile cursor
            for g, segs, first_touch, last_touch in sched:
                # one PSUM bank per window in this group
                psums = {w: psum_p.tile([128, 128], f32, tag="psum", name=f"ps_w{w}")
                         for w in first_touch}
                for ch in range(N_CHUNKS):
                    seg_tiles = sum(nt for (_, _, nt) in segs[ch])
                    if seg_tiles == 0:
                        continue
                    n_idx = seg_tiles * 128
                    msgs = msgs_p.tile([128, seg_tiles * 128], dt, tag="msgs")
                    m3 = msgs[:].rearrange("p (b f) -> p b f", f=F)
                    nc.gpsimd.dma_gather(
                        m3,
                        y_d[ch * CHUNK : (ch + 1) * CHUNK, :],
                        idx_sb[:, g_tile * 8 : g_tile * 8 + n_idx // 16],
                        n_idx,
                        n_idx,
                        F,
                        single_packet=False,
                    )
                    tt = 0
                    for wseg, q, nt in segs[ch]:
                        for k in range(nt):
                            oh = oh_p.tile([128, 128], oh_dt)
                            gt = g_tile + tt + k
                            nc.vector.tensor_scalar(
                                oh[:],
                                iota_sb[:],
                                slots_sb[:, gt : gt + 1],
                                None,
                                mybir.AluOpType.is_equal,
                            )
                            nc.tensor.matmul(
                                psums[wseg][:],
                                lhsT=msgs[:, (tt + k) * 128 : (tt + k + 1) * 128],
                                rhs=oh[:],
                                start=(first_touch[wseg] == (ch, wseg) and k == 0),
                                stop=(last_touch[wseg] == (ch, wseg) and k == nt - 1),
                            )
                        tt += nt
                    g_tile += seg_tiles
                # acc[:, window cols] += psum_w
                for w, pt in psums.items():
                    nc.vector.tensor_tensor(
                        out=acc[:, w * 128 : w * 128 + 128],
                        in0=acc[:, w * 128 : w * 128 + 128],
                        in1=pt[:],
                        op=mybir.AluOpType.add,
                    )
            assert g_tile == T

            # tail: out^T = W^T @ (u * acc) + (x^T + b)
            SL = 512
            for s0 in range(0, S, SL):
                n = min(SL, S - s0)
                sl = slice(s0, s0 + n)
                us_t = fin_p.tile([128, SL], f32, tag="us")
                xs_t = fin_p.tile([128, SL], f32, tag="xs")
                nc.sync.dma_start(us_t[:, :n], usT_d[:, sl])
                nc.sync.dma_start(xs_t[:, :n], xsT_d[:, sl])
                nc.vector.tensor_tensor(
                    out=acc[:, sl], in0=acc[:, sl], in1=us_t[:, :n],
                    op=mybir.AluOpType.mult,
                )
                pf = fpsum_p.tile([128, SL], f32)
                nc.tensor.matmul(pf[:, :n], lhsT=w_sb[:], rhs=acc[:, sl],
                                 start=True, stop=True)
                ot = fin_p.tile([128, SL], f32, tag="ot")
                nc.vector.tensor_tensor(
                    out=ot[:, :n], in0=pf[:, :n], in1=xs_t[:, :n],
                    op=mybir.AluOpType.add,
                )
                nc.sync.dma_start(out_d[:, sl], ot[:, :n])

    nc.compile()
    return nc


_PROGRAM_CACHE = {}


def _get_program(T, sched_key, sched):
    key = (T, sched_key)
    if key not in _PROGRAM_CACHE:
        _PROGRAM_CACHE[key] = _build_program(T, sched)
    return _PROGRAM_CACHE[key]


def _prepare(x, edge_index, W, b):
    x = np.asarray(x, dtype=np.float32)
    edge_index = np.asarray(edge_index)
    W = np.asarray(W, dtype=np.float32)
    b = np.asarray(b, dtype=np.float32)

    u, n_tiles, sched, T, idx16, slots = _host_plan(edge_index)

    y = np.zeros((NPAD, F), dtype=np.float32)
    y[:N_NODES] = u[:, None] * x
    y = y.astype(getattr(np, MSGS_DT) if MSGS_DT != "float32" else np.float32)

    iota = np.tile(np.arange(128, dtype=np.float32), (128, 1)).astype(
        getattr(np, "float32" if OH_DT == "float32" else OH_DT)
    )

    STOT = N_CORES * S
    u_pad = np.zeros(STOT, dtype=np.float32)
    u_pad[:N_NODES] = u
    x_pad = np.zeros((STOT, F), dtype=np.float32)
    x_pad[:N_NODES] = x
    y_pad = np.zeros((STOT, F), dtype=np.float32)
    y_pad[:N_NODES] = y[:N_NODES]

    in_maps = []
    for c in range(N_CORES):
        rows = slice(c * S, (c + 1) * S)
        # idx stream position i -> [i % 16, i // 16]; 16-row block
        # replicated 8x along partitions (one copy per Q7 core group)
        idx_c = np.tile(idx16[c].reshape(-1, 16).T, (8, 1)).copy()  # [128, T*8]
        slots_c = slots[c].reshape(T, 128).T.copy()  # [128, T]
        usT = np.repeat(u_pad[rows][None, :], 128, axis=0).copy()
        ysT = y_pad[rows].T.copy()
        xsT = (x_pad[rows] + b[None, :]).T.copy()
        in_maps.append(
            {
                "y": y,
                "idx16": idx_c,
                "slots": slots_c.astype(np.float32),
                "iota": iota,
                "usT": usT,
                "ysT": np.ascontiguousarray(ysT),
                "xsT": np.ascontiguousarray(xsT),
                "W": W,
            }
        )

    sched_key = tuple(
        (g, tuple(tuple(tuple(t) for t in seg) for seg in segs))
        for g, segs, _, _ in sched
    )
    nc = _get_program(T, sched_key, sched)
    return nc, in_maps


def _unshard(results):
    out = np.empty((N_CORES, S, F), dtype=np.float32)
    for c in range(N_CORES):
        out[c] = results[c]["outT"].T
    return out.reshape(N_CORES * S, F)[:N_NODES]


def kernel(x, edge_index, W, b):
    from concourse.bass_utils import run_bass_kernel_spmd

    nc, in_maps = _prepare(x, edge_index, W, b)
    res = run_bass_kernel_spmd(nc, in_maps, list(range(N_CORES)))
    return _unshard(res.results)


if __name__ == "__main__":
    rng = np.random.default_rng(0)
    x = rng.standard_normal((N_NODES, F), dtype=np.float32)
    ei = rng.integers(0, N_NODES, size=(2, 1600000)).astype(np.int64)
    W = rng.standard_normal((F, F), dtype=np.float32) / np.sqrt(F)
    b = np.zeros(F, dtype=np.float32)
    out = kernel(x=x, edge_index=ei, W=W, b=b)
    print(out.shape, out.dtype)
